# revision 10
# baseline (speedup 1.0000x reference)
"""BatchedLoRA trn2 kernel: out[t,n,o] = 2.0 * (x @ A[n].T) @ B[n].T.

Sharding: data-parallel over T across 8 cores (1024 tokens each); every core
computes all 8 experts for its token slab.

Per-core dataflow:
  mm1 (fp32):  adT[r_all=512, t] = contract_d(A_allT, xT)
  mm2 (fp32r): out[t, o] per expert; experts paired (2m at partitions 0-63,
       2m+1 at 64-127) so the two K=64 matmuls occupy disjoint PE row groups.
Host-side prep: transpose x/A/B, fold the 2.0 scale into B, group DRAM layout
into [128, 4096] DMA-friendly blocks.
"""
import numpy as np
from contextlib import ExitStack

from concourse import bacc, tile, mybir
from concourse.bass_utils import run_bass_kernel_spmd

# Problem dims (hardcoded per contract)
T, D, DO, R, NE = 8192, 2048, 2048, 64, 8
SCALE = 2.0
N_CORES = 8
TC = T // N_CORES          # tokens per core = 1024
P = 128
KT = D // P                # 16 d-tiles
TCH = TC // 512            # 2 t-chunks of 512 (mm1 moving dim)
NP = NE // 2               # 4 expert pairs
TT = TC // P               # 8 t-chunks of 128 (mm2 stationary dim)
OC = DO // 512             # 4 o-chunks of 512 (mm2 moving dim)
RA = NE * R                # 512 ranks across experts

F32 = mybir.dt.float32
F32R = mybir.dt.float32r


def build_nc(reps: int = 1, variant: str = "sgring"):
    """Per-core bass program. reps>1 repeats the body for differential timing.

    variant selects DMA-ring placement / stage ablations used during tuning.
    "sgring" (default, fastest measured): output DMAs on the sync HWDGE ring +
    the gpsimd SWDGE ring, keeping the scalar(ACT) sequencer free for PSUM
    evictions. Ablations: full | 3ring | sring | noout | nomm2 | nomm1 |
    inonly | ps6."""
    nc = bacc.Bacc("TRN2", target_bir_lowering=False, debug=False)
    x4_ap = nc.dram_tensor("x4", [4, P, 4096], F32, kind="ExternalInput").ap()
    a2_ap = nc.dram_tensor("a2", [2, P, 4096], F32, kind="ExternalInput").ap()
    b2_ap = nc.dram_tensor("b2", [2, P, 4096], F32, kind="ExternalInput").ap()
    out_ap = nc.dram_tensor("out", [TC, NE, DO], F32, kind="ExternalOutput").ap()

    dma_engines = [nc.sync, nc.scalar]
    out_engines = {
        "3ring": [nc.sync, nc.scalar, nc.gpsimd],
        "sgring": [nc.sync, nc.gpsimd],
        "sring": [nc.sync],
        "tring": [nc.sync, nc.tensor],
        "4ring": [nc.sync, nc.scalar, nc.gpsimd, nc.tensor],
    }.get(variant, [nc.sync, nc.scalar])

    with tile.TileContext(nc) as tc, ExitStack() as ctx:
        xr_p = ctx.enter_context(tc.tile_pool(name="xr", bufs=1))
        ar_p = ctx.enter_context(tc.tile_pool(name="ar", bufs=1))
        bs_p = ctx.enter_context(tc.tile_pool(name="bs", bufs=1))
        br_p = ctx.enter_context(tc.tile_pool(name="br", bufs=1))
        ad_p = ctx.enter_context(tc.tile_pool(name="ad", bufs=1))
        ps1_p = ctx.enter_context(tc.tile_pool(name="ps1", bufs=2, space="PSUM"))
        ps2_p = ctx.enter_context(tc.tile_pool(
            name="ps2", bufs=6 if variant == "ps6" else 4, space="PSUM"))
        os_p = ctx.enter_context(tc.tile_pool(name="os", bufs=4))

        xr = xr_p.tile([P, KT * TC], F32)        # 64KB/part, fp32 resident
        ar = ar_p.tile([P, KT * RA], F32)        # 32KB/part
        br = br_p.tile([P, NP * DO], F32R)       # 32KB/part
        ad = ad_p.tile([P, NP * TC], F32R)       # 16KB/part

        if variant == "nomm1":
            nc.any.memset(ad[:].bitcast(F32), 0.0)
        ev = 0
        for rep in range(reps):
            # ---- input loads (alternate HWDGE rings); B last, it's only
            # needed for mm2 ----
            for g in range(4):
                dma_engines[g % 2].dma_start(
                    xr[:, g * 4096:(g + 1) * 4096], x4_ap[g, :, :])
            for g in range(2):
                dma_engines[1 - g % 2].dma_start(
                    ar[:, g * 4096:(g + 1) * 4096], a2_ap[g, :, :])
            for g in range(2):
                bs = bs_p.tile([P, 4096], F32, tag="bs", name=f"bs{rep}_{g}")
                dma_engines[g % 2].dma_start(bs[:], b2_ap[g, :, :])
                nc.vector.tensor_copy(br[:, g * 4096:(g + 1) * 4096], bs[:])

            if variant == "inonly":
                ot = os_p.tile([P, 2048], F32, tag="os", name=f"mark{rep}")
                nc.vector.tensor_copy(ot[:], xr[:, :2048])
                nc.sync.dma_start(out_ap[0:P, 0, :], ot[:])
                continue

            for m in range(NP):
                # ---- mm1 (fp32): adT pair m = [128 r, 1024 t] ----
                for tch in range(TCH if variant != "nomm1" else 0):
                    ps = ps1_p.tile([P, 512], F32, tag="ps1",
                                    name=f"ps1_{rep}_{m}_{tch}")
                    for k in range(KT):
                        nc.tensor.matmul(
                            ps[:],
                            ar[:, k * RA + m * P: k * RA + (m + 1) * P],
                            xr[:, k * TC + tch * 512: k * TC + (tch + 1) * 512],
                            start=(k == 0), stop=(k == KT - 1))
                    nc.vector.tensor_copy(
                        ad[:, m * TC + tch * 512: m * TC + (tch + 1) * 512],
                        ps[:])

                if variant == "nomm2":
                    ot = os_p.tile([P, 2048], F32, tag="os",
                                   name=f"mk{rep}_{m}")
                    nc.vector.tensor_copy(
                        ot[:, :TC], ad[:, m * TC:(m + 1) * TC].bitcast(F32))
                    nc.sync.dma_start(out_ap[0:P, m, :], ot[:])
                    continue

                # ---- mm2 (fp32r), experts 2m / 2m+1 ----
                for tt in range(TT):
                    oth = [os_p.tile([P, DO], F32, tag="os",
                                     name=f"os{rep}_{m}_{tt}_{h}")
                           for h in range(2)]
                    for oc in range(OC):
                        for half in range(2):
                            ps = ps2_p.tile([P, 512], F32, tag="ps2",
                                            name=f"ps2_{rep}_{m}_{tt}_{oc}_{half}")
                            nc.tensor.matmul(
                                ps[:],
                                ad[half * 64:(half + 1) * 64,
                                   m * TC + tt * P: m * TC + (tt + 1) * P],
                                br[half * 64:(half + 1) * 64,
                                   m * DO + oc * 512: m * DO + (oc + 1) * 512],
                                start=True, stop=True)
                            dst = oth[half][:, oc * 512:(oc + 1) * 512]
                            if ev % 2 == 0:
                                nc.vector.tensor_copy(dst, ps[:])
                            else:
                                nc.scalar.mul(dst, ps[:], 1.0)
                            ev += 1
                    if variant == "noout" and tt != 0:
                        continue
                    for half in range(2):
                        eng = out_engines[(2 * (tt * NP + m) + half)
                                          % len(out_engines)]
                        eng.dma_start(
                            out_ap[tt * P:(tt + 1) * P, 2 * m + half, :],
                            oth[half][:])
    nc.finalize()
    return nc


def make_in_maps(x, A_weights, B_weights):
    xT = np.ascontiguousarray(x.T)                             # [D, T]
    aT = np.ascontiguousarray(A_weights.reshape(RA, D).T)      # [D, 512]
    b2 = (SCALE * B_weights).transpose(0, 2, 1)                # [NE, R, DO]
    bp = b2.reshape(NP, P, DO)                                 # expert pairs

    a2 = aT.reshape(KT, P, RA).transpose(1, 0, 2).reshape(P, KT * RA)
    a2 = np.ascontiguousarray(
        a2.reshape(P, 2, 4096).transpose(1, 0, 2))             # [2, 128, 4096]
    b2g = np.ascontiguousarray(
        bp.reshape(2, 2, P, DO).transpose(0, 2, 1, 3).reshape(2, P, 4096))

    in_maps = []
    for c in range(N_CORES):
        xc = xT[:, c * TC:(c + 1) * TC]                        # [2048, 1024]
        x4 = xc.reshape(KT, P, TC).transpose(1, 0, 2).reshape(P, KT * TC)
        x4 = np.ascontiguousarray(
            x4.reshape(P, 4, 4096).transpose(1, 0, 2))         # [4, 128, 4096]
        in_maps.append({"x4": x4, "a2": a2, "b2": b2g})
    return in_maps


_NC_CACHE = {}


def kernel(x, A_weights, B_weights):
    x = np.asarray(x, dtype=np.float32)
    A_weights = np.asarray(A_weights, dtype=np.float32)
    B_weights = np.asarray(B_weights, dtype=np.float32)
    if "nc" not in _NC_CACHE:
        _NC_CACHE["nc"] = build_nc(reps=1)
    nc = _NC_CACHE["nc"]
    in_maps = make_in_maps(x, A_weights, B_weights)
    res = run_bass_kernel_spmd(nc, in_maps, list(range(N_CORES)))
    return np.concatenate([res.results[c]["out"] for c in range(N_CORES)], axis=0)
